# revision 13
# baseline (speedup 1.0000x reference)
"""BiLSTM-CRF loss on 8 Trainium2 NeuronCores.

Strategy:
  - Direction-split: cores 0-3 run the forward LSTM, cores 4-7 the backward
    LSTM (on host-pre-flipped input). Within each group the batch (32) is
    sharded 4 ways -> 8 sequences per core.
  - Device kernel A: input projections x @ W_ih.T + (b_ih+b_hh) as one big
    matmul per core (bias folded in via a ones-row matmul).
  - Device kernel B: 64 unrolled LSTM recurrence steps (compiled once, called
    8x with c/hT state roundtrip). Recurrent matmul is lhsT=h.T (tiny
    stationary), rhs=W_hh.T resident in SBUF; x-projection is folded into the
    same PSUM accumulation group via an identity-stationary matmul.
  - Host (numpy): embedding gather, sequence flips, emissions, CRF
    forward/gold score (cheap, O(T*B*L^2)).
"""
import sys
import numpy as np

sys.path.insert(0, '/opt/trn_rl_repo')

import concourse.bacc as bacc
import concourse.mybir as mybir
from concourse.tile import TileContext
from concourse.bass_utils import run_bass_kernel_spmd
import ml_dtypes

BF16 = ml_dtypes.bfloat16
F32 = np.float32

B, T = 32, 512
V, D, L = 50257, 512, 48
G = 4 * D  # 2048 gate width
NCORES = 8
BL = 8       # sequences per core (dir-split: 4 cores x 8 = 32 per direction)
CH = 128     # recurrence steps per kernel-B invocation
NCH = T // CH
NTOK = T * BL  # tokens per core = 4096
MT = NTOK // 128  # M-tiles in projection = 32

_SIG = mybir.ActivationFunctionType.Sigmoid
_TANH = mybir.ActivationFunctionType.Tanh

_cache = {}


def _build_proj():
    nc = bacc.Bacc()
    dt = mybir.dt
    embT = nc.declare_dram_parameter("embT", [128, 4 * NTOK], dt.bfloat16, isOutput=False)
    wih = nc.declare_dram_parameter("wih", [128, 4 * G], dt.bfloat16, isOutput=False)
    bias = nc.declare_dram_parameter("bias", [1, G], dt.bfloat16, isOutput=False)
    ones = nc.declare_dram_parameter("ones", [1, 128], dt.bfloat16, isOutput=False)
    xp = nc.declare_dram_parameter("xp", [MT, 128, G], dt.bfloat16, isOutput=True)

    with TileContext(nc) as tc:
        with (
            tc.tile_pool(name="const", bufs=1) as cpool,
            tc.tile_pool(name="psum", bufs=2, space="PSUM") as ppool,
            tc.tile_pool(name="out", bufs=3) as opool,
        ):
            embT_sb = cpool.tile([128, 4 * NTOK], dt.bfloat16)
            wih_sb = cpool.tile([128, 4 * G], dt.bfloat16)
            bias_sb = cpool.tile([1, G], dt.bfloat16)
            ones_sb = cpool.tile([1, 128], dt.bfloat16)
            nc.sync.dma_start(out=embT_sb[:], in_=embT[:])
            nc.sync.dma_start(out=wih_sb[:], in_=wih[:])
            nc.sync.dma_start(out=bias_sb[:], in_=bias[:])
            nc.sync.dma_start(out=ones_sb[:], in_=ones[:])
            for m in range(MT):
                ps = ppool.tile([128, G], dt.float32)
                for nb in range(4):
                    o = ps[:, nb * 512:(nb + 1) * 512]
                    for kc in range(4):
                        nc.tensor.matmul(
                            o,
                            embT_sb[:, kc * NTOK + m * 128: kc * NTOK + (m + 1) * 128],
                            wih_sb[:, kc * G + nb * 512: kc * G + (nb + 1) * 512],
                            start=(kc == 0), stop=False)
                    nc.tensor.matmul(
                        o, ones_sb[0:1, :], bias_sb[0:1, nb * 512:(nb + 1) * 512],
                        start=False, stop=True)
                ot = opool.tile([128, G], dt.bfloat16)
                nc.vector.tensor_copy(ot[:], ps[:])
                nc.sync.dma_start(out=xp[m], in_=ot[:])
    nc.finalize()
    return nc


def _build_rec():
    nc = bacc.Bacc()
    dt = mybir.dt
    xpc = nc.declare_dram_parameter("xpc", [CH, BL, G], dt.bfloat16, isOutput=False)
    whh = nc.declare_dram_parameter("whh", [128, 4 * G], dt.bfloat16, isOutput=False)
    i8 = nc.declare_dram_parameter("i8", [8, 8], dt.bfloat16, isOutput=False)
    c_in = nc.declare_dram_parameter("c_in", [BL, D], dt.float32, isOutput=False)
    hT_in = nc.declare_dram_parameter("hT_in", [128, 4 * BL], dt.bfloat16, isOutput=False)
    hs = nc.declare_dram_parameter("hs", [CH, BL, D], dt.bfloat16, isOutput=True)
    c_out = nc.declare_dram_parameter("c_out", [BL, D], dt.float32, isOutput=True)
    hT_out = nc.declare_dram_parameter("hT_out", [128, 4 * BL], dt.bfloat16, isOutput=True)

    with TileContext(nc) as tc:
        with (
            tc.tile_pool(name="const", bufs=1) as cpool,
            tc.tile_pool(name="xp", bufs=3) as xpool,
            tc.tile_pool(name="state", bufs=2) as spool,
            tc.tile_pool(name="gates", bufs=2) as gpool,
            tc.tile_pool(name="h", bufs=3) as hpool,
            tc.tile_pool(name="pg", bufs=1, space="PSUM") as pgpool,
            tc.tile_pool(name="pt", bufs=2, space="PSUM") as ptpool,
        ):
            whh_sb = cpool.tile([128, 4 * G], dt.bfloat16)
            i8_sb = cpool.tile([8, 8], dt.bfloat16)
            nc.sync.dma_start(out=whh_sb[:], in_=whh[:])
            nc.sync.dma_start(out=i8_sb[:], in_=i8[:])
            c_prev = spool.tile([BL, D], dt.float32, tag="c")
            nc.sync.dma_start(out=c_prev[:], in_=c_in[:])
            hT_prev = spool.tile([128, 4 * BL], dt.bfloat16, tag="hT")
            nc.sync.dma_start(out=hT_prev[:], in_=hT_in[:])

            for j in range(CH):
                xp_sb = xpool.tile([BL, G], dt.bfloat16, tag="xp")
                nc.sync.dma_start(out=xp_sb[:], in_=xpc[j])
                # per-bank PSUM tiles: gate activations start as soon as
                # their own bank's accumulation group finishes
                pgs = [pgpool.tile([BL, 512], dt.float32, tag=f"pg{nb}",
                                   name=f"pg{nb}") for nb in range(4)]
                for nb in range(4):
                    nc.tensor.matmul(
                        pgs[nb][:], i8_sb[:],
                        xp_sb[:, nb * 512:(nb + 1) * 512], start=True, stop=False)
                acts = []
                for nb in range(4):
                    for kc in range(4):
                        nc.tensor.matmul(
                            pgs[nb][:], hT_prev[:, kc * BL:(kc + 1) * BL],
                            whh_sb[:, kc * G + nb * 512: kc * G + (nb + 1) * 512],
                            start=False, stop=(kc == 3))
                    a_sb = gpool.tile([BL, D], dt.bfloat16, tag=f"act{nb}",
                                      name=f"act{nb}")
                    nc.scalar.activation(a_sb[:], pgs[nb][:],
                                         _TANH if nb == 2 else _SIG)
                    acts.append(a_sb)
                i_sb, f_sb, g_sb, o_sb = acts
                fc = gpool.tile([BL, D], dt.float32, tag="fc")
                nc.vector.tensor_mul(fc[:], f_sb[:], c_prev[:])
                ig = gpool.tile([BL, D], dt.float32, tag="ig")
                nc.vector.tensor_mul(ig[:], i_sb[:], g_sb[:])
                c_new = spool.tile([BL, D], dt.float32, tag="c")
                nc.vector.tensor_add(c_new[:], ig[:], fc[:])
                tc_sb = gpool.tile([BL, D], dt.bfloat16, tag="tc")
                nc.scalar.activation(tc_sb[:], c_new[:], _TANH)
                h_sb = hpool.tile([BL, D], dt.bfloat16, tag="h")
                nc.vector.tensor_mul(h_sb[:], o_sb[:], tc_sb[:])
                nc.sync.dma_start(out=hs[j], in_=h_sb[:])
                pt = ptpool.tile([128, 4 * BL], dt.bfloat16, tag="pt")
                for kc in range(4):
                    nc.tensor.transpose(
                        pt[:, kc * BL:(kc + 1) * BL],
                        h_sb[:, kc * 128:(kc + 1) * 128], i8_sb[:])
                hT_new = spool.tile([128, 4 * BL], dt.bfloat16, tag="hT")
                nc.vector.tensor_copy(hT_new[:], pt[:])
                c_prev, hT_prev = c_new, hT_new
            nc.sync.dma_start(out=c_out[:], in_=c_prev[:])
            nc.sync.dma_start(out=hT_out[:], in_=hT_prev[:])
    nc.finalize()
    return nc


def _chunk128(a):
    """[512, N] -> [128, 4*N] with k-chunk kc at cols [kc*N:(kc+1)*N]."""
    n = a.shape[1]
    return np.ascontiguousarray(
        a.reshape(4, 128, n).transpose(1, 0, 2).reshape(128, 4 * n))


def _seq_flip(x, lengths):
    t = np.arange(x.shape[1])[None, :]
    idx = lengths[:, None] - 1 - t
    idx = np.where(idx >= 0, idx, t)
    return np.take_along_axis(x, idx[:, :, None], axis=1)


def _logsumexp(a, axis):
    m = np.max(a, axis=axis, keepdims=True)
    return np.squeeze(m, axis) + np.log(np.sum(np.exp(a - m), axis=axis))


def kernel(tokens, tags, lengths, embed, W_ih_f, W_hh_f, b_ih_f, b_hh_f,
           W_ih_b, W_hh_b, b_ih_b, b_hh_b, init_hidden, W_emit, b_emit,
           start_trans, trans, end_trans):
    tokens = np.asarray(tokens).astype(np.int64)
    tags = np.asarray(tags).astype(np.int64)
    lengths = np.asarray(lengths).astype(np.int64)
    embed = np.asarray(embed, F32)

    if "proj" not in _cache:
        _cache["proj"] = _build_proj()
        _cache["rec"] = _build_rec()
    nc_p, nc_r = _cache["proj"], _cache["rec"]

    emb = embed[tokens]                      # [B,T,D] f32
    embr = _seq_flip(emb, lengths)           # reversed input for bwd lstm

    # ---- per-core packing ----
    ones = np.ones((1, 128), BF16)
    i8 = np.eye(8, dtype=BF16)
    wih_pc, bias_pc, whh_pc, hT0_pc, c0_pc, emb_pc = [], [], [], [], [], []
    for c in range(NCORES):
        d = 0 if c < 4 else 1
        W_ih, W_hh = (W_ih_f, W_hh_f) if d == 0 else (W_ih_b, W_hh_b)
        bsum = (np.asarray(b_ih_f) + np.asarray(b_hh_f)) if d == 0 else \
               (np.asarray(b_ih_b) + np.asarray(b_hh_b))
        wih_pc.append(_chunk128(np.asarray(W_ih, F32).T).astype(BF16))
        whh_pc.append(_chunk128(np.asarray(W_hh, F32).T).astype(BF16))
        bias_pc.append(np.asarray(bsum, F32).reshape(1, G).astype(BF16))
        h0 = np.asarray(init_hidden, F32)[d]          # [D]
        hT0 = np.broadcast_to(h0[:, None], (D, BL))   # [D, BL]
        hT0_pc.append(_chunk128(hT0).astype(BF16))
        c0_pc.append(np.broadcast_to(h0[None, :], (BL, D)).astype(F32).copy())
        x = emb if d == 0 else embr
        sl = x[(c % 4) * BL:(c % 4 + 1) * BL]         # [BL, T, D]
        # [D, T, BL] -> [D, T*BL] (t-major, b-minor) -> chunked
        embT = sl.transpose(2, 1, 0).reshape(D, NTOK)
        emb_pc.append(_chunk128(embT).astype(BF16))

    # ---- projections on device ----
    in_maps = [dict(embT=emb_pc[c], wih=wih_pc[c], bias=bias_pc[c], ones=ones)
               for c in range(NCORES)]
    res = run_bass_kernel_spmd(nc_p, in_maps, core_ids=list(range(NCORES)))
    # xp [MT,128,G] -> [T, BL, G]
    xp_pc = [r["xp"].reshape(T, BL, G) for r in res.results]

    # ---- recurrence: NCH sequential chunk calls ----
    hs_pc = [np.empty((T, BL, D), BF16) for _ in range(NCORES)]
    c_st, hT_st = c0_pc, hT0_pc
    for k in range(NCH):
        in_maps = [dict(xpc=np.ascontiguousarray(xp_pc[c][k * CH:(k + 1) * CH]),
                        whh=whh_pc[c], i8=i8, c_in=c_st[c], hT_in=hT_st[c])
                   for c in range(NCORES)]
        res = run_bass_kernel_spmd(nc_r, in_maps, core_ids=list(range(NCORES)))
        for c in range(NCORES):
            hs_pc[c][k * CH:(k + 1) * CH] = res.results[c]["hs"]
        c_st = [res.results[c]["c_out"] for c in range(NCORES)]
        hT_st = [res.results[c]["hT_out"] for c in range(NCORES)]

    # ---- host epilogue ----
    hf = np.concatenate([hs_pc[c].astype(F32) for c in range(4)], axis=1)   # [T,32,D]
    hbr = np.concatenate([hs_pc[c].astype(F32) for c in range(4, 8)], axis=1)
    hf = hf.transpose(1, 0, 2)            # [B,T,D]
    hb = _seq_flip(hbr.transpose(1, 0, 2), lengths)
    feats = np.concatenate([hf, hb], axis=-1)          # [B,T,2D]
    emissions = feats @ np.asarray(W_emit, F32).T + np.asarray(b_emit, F32)

    e = emissions.astype(np.float64)
    tr = np.asarray(trans, np.float64)
    st = np.asarray(start_trans, np.float64)
    et = np.asarray(end_trans, np.float64)
    mask = np.arange(T)[None, :] < lengths[:, None]
    alpha = e[:, 0] + st
    expTrT = np.exp(tr).T  # [j, i]: new_i = LSE_j(alpha_j + tr[i,j])
    for t in range(1, T):
        m = alpha.max(axis=1, keepdims=True)
        new = e[:, t] + m + np.log(np.exp(alpha - m) @ expTrT)
        alpha = np.where(mask[:, t][:, None], new, alpha)
    fwd = _logsumexp(alpha + et, axis=-1)
    e_tag = np.take_along_axis(e, tags[..., None], axis=-1)[..., 0]
    step_scores = tr[tags[:, 1:], tags[:, :-1]] + e_tag[:, 1:]
    last_tag = np.take_along_axis(tags, (lengths - 1)[:, None], axis=1)[:, 0]
    gold = (st[tags[:, 0]] + e_tag[:, 0]
            + np.sum(np.where(mask[:, 1:], step_scores, 0.0), axis=-1)
            + et[last_tag])
    return np.float32(np.sum(fwd - gold))
